# revision 22
# baseline (speedup 1.0000x reference)
"""Trainium2 Bass kernel for the DPMM variational update (nn_DPMM_6828998000885).

Strategy
--------
Every statistic the reference needs over the data is a block of ONE augmented
weighted Gram matrix:

    G[t, i, j] = sum_n  Xaug[n, i] * Phi[n, t] * Xaug[n, j]

with Xaug = [sigmoid(x@W + b) (40 cols) | ones (1) | inputData (8) | y (1)]
(50 columns).  Blocks:
    A[t]    = G[t, 0:41, 0:41]      (X^T diag(phi_t) X, X = [hmap, 1])
    b[t]    = G[t, 0:41, 49]        (X^T diag(phi_t) y)
    c[t]    = G[t, 49, 49]          (sum phi_t y^2)
    S[.,.,t]= G[t, 41:49, 41:49]    (x^T diag(phi_t) x)
    t12     = G[t, 40, 41:49]       (x^T Phi)
    sumPhi  = G[t, 40, 40]

The E-step residual term needs no second pass over N because
    t1[t] = sum_n phi (y - x.w)^2 = c[t] - 2 w.b[t] + w^T A[t] w.

Device kernel (SPMD over 8 cores, N sharded): per 128-row chunk build the
row-wise Khatri-Rao product W[n, t*50+j] = Phi[n,t]*Xaug[n,j] on the vector
engine (broadcast access patterns), and accumulate
Stats[i, t*50+j] += Xaug_chunk^T @ W_chunk on the tensor engine into PSUM.
The tiny [T,41,41] inverses and all O(T*F^2) math run on the host in float64.
"""

import numpy as np

# Problem constants (hardcoded per the harness contract).
N, T, D, H = 100000, 16, 8, 40
HP1 = H + 1          # 41
F = HP1 + D + 1      # 50 Xaug columns: [hmap(40) | 1 | x(8) | y(1)]
NCORES = 8
NPC = N // NCORES    # 12500 rows per core
P = 128
NCHUNK = 100                  # padded to a multiple of DMA_BATCH
NPAD = NCHUNK * P             # 12800 (zero-padded; phi=0 rows contribute 0)
WCOLS = T * F                 # 800
HALF = WCOLS // 2             # 400 (PSUM bank limit is 512 f32 of free dim)

ALPHA_DP = 1.0
KAPPA0 = 1000.0
NU0 = 100.0
PSI0_SCALE = 500.0

TRACE = False          # set by test harness to capture an NTFF profile
LAST_RESULTS = None    # BassKernelResults of the last run (for the harness)

_CACHE = {}


DMA_BATCH = 10  # chunks per phi DMA (fewer DIRECT2D descriptors on SP seq)
ACT_T = 2       # Khatri-Rao t-blocks built by ScalarE instead of VectorE
TV = T - ACT_T  # t-blocks built on VectorE


def _build_bass(nchunk=NCHUNK, use_f32r=False):
    import concourse.tile as tile
    from concourse import bacc, mybir
    from contextlib import ExitStack

    f32 = mybir.dt.float32
    f32r = mybir.dt.float32r
    npad = nchunk * P
    assert nchunk % DMA_BATCH == 0 and nchunk % 2 == 0

    nc = bacc.Bacc("TRN2", debug=False, num_devices=NCORES)
    wdt = f32r if use_f32r else f32
    # bigc = [xaug(50) | phi(16)] per row; xaug = [hmap(40) | 1 | x(8) | y]
    # (ELM features precomputed on host). Host pre-permutes rows into
    # batch-contiguous layout [nbatch*128, DMA_BATCH*66] so each batch DMA is
    # one dense per-partition-contiguous burst.
    nbatch = nchunk // DMA_BATCH
    bigc_d = nc.dram_tensor("bigc", [nbatch * P, DMA_BATCH * (F + T)], f32,
                            kind="ExternalInput").ap()
    # accumulators: [A|B] x [even|odd PE column-group]
    out_d = nc.dram_tensor("stats", [F, 2 * WCOLS], f32,
                           kind="ExternalOutput").ap()

    with ExitStack() as ctx:
        tc = ctx.enter_context(tile.TileContext(nc))
        comb_pool = ctx.enter_context(tc.tile_pool(name="comb", bufs=5))
        w_pool = ctx.enter_context(tc.tile_pool(name="w", bufs=10))
        stats_pool = ctx.enter_context(tc.tile_pool(name="st", bufs=1,
                                                    space="PSUM"))
        out_pool = ctx.enter_context(tc.tile_pool(name="outp", bufs=1))

        # rows 0:50 accumulate even chunks (col-group 0), rows 64:114 odd
        # chunks (col-group 2) -- matmuls on distinct column-groups stream
        # concurrently through separate XBUSes.
        statsA = stats_pool.tile([64 + F, HALF], f32)
        statsB = stats_pool.tile([64 + F, HALF], f32)

        comb_t = None
        for c in range(nchunk):
            r0 = c * P
            ci = c % DMA_BATCH
            if ci == 0:
                b = c // DMA_BATCH
                comb_t = comb_pool.tile([P, DMA_BATCH, F + T], f32,
                                        tag="comb")
                nc.sync.dma_start(
                    comb_t[:],
                    bigc_d[b * P:(b + 1) * P, :].rearrange(
                        "p (c f) -> p c f", c=DMA_BATCH),
                )

            xa = comb_t[:, ci, 0:F]
            w_t = w_pool.tile([P, WCOLS], wdt)
            # W[n, t*F + j] = Phi[n, t] * Xaug[n, j]
            nc.vector.tensor_mul(
                w_t[:, 0:TV * F].rearrange("p (t f) -> p t f", t=TV),
                xa[:, None, :].broadcast_to([P, TV, F]),
                comb_t[:, ci, F:F + TV][:, :, None].broadcast_to([P, TV, F]),
            )
            for k in range(ACT_T):
                t_idx = TV + k
                nc.scalar.activation(
                    w_t[:, t_idx * F:(t_idx + 1) * F],
                    xa,
                    mybir.ActivationFunctionType.Copy,
                    scale=comb_t[:, ci, F + t_idx:F + t_idx + 1],
                )

            first, last = c < 2, c >= nchunk - 2
            g = (c % 2) * 64
            nc.tensor.matmul(statsA[g:g + F, :], xa, w_t[:, 0:HALF],
                             start=first, stop=last, tile_position=(0, g))
            nc.tensor.matmul(statsB[g:g + F, :], xa, w_t[:, HALF:WCOLS],
                             start=first, stop=last, tile_position=(0, g))

        out_t = out_pool.tile([F, 2 * WCOLS], f32)
        for gi in range(2):
            g = gi * 64
            nc.scalar.copy(out_t[:, gi * WCOLS:gi * WCOLS + HALF],
                           statsA[g:g + F, :])
            nc.scalar.copy(out_t[:, gi * WCOLS + HALF:(gi + 1) * WCOLS],
                           statsB[g:g + F, :])
        nc.sync.dma_start(out_d, out_t[:])

    nc.compile()
    return nc


def _get_bass():
    if "nc" not in _CACHE:
        _CACHE["nc"] = _build_bass()
    return _CACHE["nc"]


def _host_xaug(X, y, Wel, bel, npc, npad):
    """[hmap | 1 | x | y] in float64 -> float32 (hmap better than device ACT)."""
    Z = X.astype(np.float64) @ Wel.astype(np.float64) + \
        bel.reshape(1, H).astype(np.float64)
    hmap = 1.0 / (1.0 + np.exp(-Z))
    xa = np.zeros((npad, F), np.float32)
    xa[:npc, 0:H] = hmap.astype(np.float32)
    xa[:npc, H] = 1.0
    xa[:npc, H + 1:H + 1 + D] = X
    xa[:npc, F - 1] = y[:, 0]
    return xa


def _batch_layout(bigc):
    """[NPAD, 66] row-major -> [nbatch*128, DMA_BATCH*66] batch-contiguous."""
    nb = NCHUNK // DMA_BATCH
    fw = bigc.shape[1]
    out = bigc.reshape(nb, DMA_BATCH, P, fw).transpose(0, 2, 1, 3)
    return np.ascontiguousarray(out.reshape(nb * P, DMA_BATCH * fw))


def _make_in_maps(Phi, X, y, Wel, bel):
    in_maps = []
    for c in range(NCORES):
        sl = slice(c * NPC, (c + 1) * NPC)
        bigc = np.zeros((NPAD, F + T), np.float32)
        bigc[:, 0:F] = _host_xaug(X[sl], y[sl], Wel, bel, NPC, NPAD)
        bigc[:NPC, F:F + T] = Phi[sl]
        in_maps.append({"bigc": _batch_layout(bigc)})
    return in_maps


def _postprocess(G, epsilonA, epsilonB, zetaA, zetaB):
    """G: [T, F, F] float64 gathered Gram stats. Returns the 12-tuple."""
    epsA = np.asarray(epsilonA, np.float64)
    epsB = np.asarray(epsilonB, np.float64)
    zetA = np.asarray(zetaA, np.float64)
    zetB = np.asarray(zetaB, np.float64)

    sumPhi = G[:, H, H]                                   # [T]
    t12 = G[:, H, HP1:HP1 + D].T                          # [D, T]
    S = np.transpose(G[:, HP1:HP1 + D, HP1:HP1 + D], (1, 2, 0))  # [D,D,T]
    A = G[:, 0:HP1, 0:HP1]                                # [T,41,41]
    b = G[:, 0:HP1, F - 1]                                # [T,41]
    cst = G[:, F - 1, F - 1]                              # [T]

    BetaGamma1_new = (1.0 + sumPhi)[None, :]
    rev_incl = np.cumsum(sumPhi[::-1])[::-1]
    BetaGamma2 = (ALPHA_DP + rev_incl - sumPhi)[None, :]

    kappa = KAPPA0 + sumPhi                               # [T]
    mu = t12 / kappa[None, :]                             # [D,T]
    nu = (sumPhi + NU0)[None, :]
    psi = (PSI0_SCALE * np.eye(D)[:, :, None]
           + S
           - kappa[None, None, :] * mu[:, None, :] * mu[None, :, :])

    epsExp = (epsA / epsB)[0]                             # [T]
    zetaExp = (zetA / zetB)[0]                            # [T]
    Mm = epsExp[:, None, None] * A + zetaExp[:, None, None] * np.eye(HP1)
    WS = np.linalg.inv(Mm)                                # [T,41,41]
    WMv = zetaExp[:, None] * np.einsum('tij,tj->ti', WS, b)
    WM = WMv[:, :, None]

    zetaA_new = zetA + (H + 1) * 0.5
    zetaB_new = zetB + 0.5 * (np.sum(WMv * WMv, axis=1)
                              + np.trace(WS, axis1=1, axis2=2))[None, :]
    epsA_new = epsA + 0.5 * sumPhi[None, :]

    t1 = cst - 2.0 * np.sum(b * WMv, axis=1) + np.einsum('ti,tij,tj->t', WMv, A, WMv)
    t2 = np.einsum('tij,tji->t', A, WS)
    epsB_new = epsB + 0.5 * (t1 + t2)[None, :]

    f = lambda a: np.ascontiguousarray(np.asarray(a, np.float32))
    return (f(BetaGamma1_new), f(BetaGamma2), f(mu), f(kappa[None, :]), f(nu),
            f(psi), f(WM), f(WS), f(zetaA_new), f(zetaB_new), f(epsA_new),
            f(epsB_new))


def _ensure_ntff_hook():
    """The axon container's antenv stub lacks axon_hooks; synthesize it and
    install the ctypes NTFF profiling hook so trace=True works. Only used
    when TRACE is set (perf measurement); best-effort."""
    import sys
    import types
    try:
        import antenv.axon_hooks  # noqa: F401
        return
    except ImportError:
        pass
    try:
        import antenv
        mod = types.ModuleType("antenv.axon_hooks")
        _state = {"hook": None}
        mod.set_axon_ntff_profile_hook = lambda h: _state.__setitem__("hook", h)
        mod.get_axon_ntff_profile_hook = lambda: _state["hook"]
        sys.modules["antenv.axon_hooks"] = mod
        antenv.axon_hooks = mod
        from trn_agent_boot.trn_boot import _ntff_profile_via_ctypes
        hook = _ntff_profile_via_ctypes('/opt/axon/libaxon_pjrt.so')
        if hook is not None:
            mod.set_axon_ntff_profile_hook(hook)
    except Exception:
        pass


def kernel(BetaGamma1, Phi, inputData, outputData, weightELM, biasELM,
           epsilonA, epsilonB, zetaA, zetaB):
    global LAST_RESULTS
    if TRACE:
        _ensure_ntff_hook()
    from concourse.bass_utils import run_bass_kernel_spmd

    Phi = np.ascontiguousarray(np.asarray(Phi, np.float32))
    X = np.ascontiguousarray(np.asarray(inputData, np.float32))
    y = np.ascontiguousarray(np.asarray(outputData, np.float32))
    Wel = np.asarray(weightELM, np.float32)
    bel = np.asarray(biasELM, np.float32)

    nc = _get_bass()
    in_maps = _make_in_maps(Phi, X, y, Wel, bel)
    res = run_bass_kernel_spmd(nc, in_maps, core_ids=list(range(NCORES)),
                               trace=TRACE)
    LAST_RESULTS = res

    stats = np.zeros((F, WCOLS), np.float64)
    for r in res.results:
        s = r["stats"].astype(np.float64)       # [F, 2*WCOLS]: two col-groups
        stats += s[:, :WCOLS] + s[:, WCOLS:]
    G = stats.reshape(F, T, F).transpose(1, 0, 2)         # [t, i, j]

    return _postprocess(G, epsilonA, epsilonB, zetaA, zetaB)


# revision 23
# speedup vs baseline: 1.0182x; 1.0182x over previous
"""Trainium2 Bass kernel for the DPMM variational update (nn_DPMM_6828998000885).

Strategy
--------
Every statistic the reference needs over the data is a block of ONE augmented
weighted Gram matrix:

    G[t, i, j] = sum_n  Xaug[n, i] * Phi[n, t] * Xaug[n, j]

with Xaug = [sigmoid(x@W + b) (40 cols) | ones (1) | inputData (8) | y (1)]
(50 columns).  Blocks:
    A[t]    = G[t, 0:41, 0:41]      (X^T diag(phi_t) X, X = [hmap, 1])
    b[t]    = G[t, 0:41, 49]        (X^T diag(phi_t) y)
    c[t]    = G[t, 49, 49]          (sum phi_t y^2)
    S[.,.,t]= G[t, 41:49, 41:49]    (x^T diag(phi_t) x)
    t12     = G[t, 40, 41:49]       (x^T Phi)
    sumPhi  = G[t, 40, 40]

The E-step residual term needs no second pass over N because
    t1[t] = sum_n phi (y - x.w)^2 = c[t] - 2 w.b[t] + w^T A[t] w.

Device kernel (SPMD over 8 cores, N sharded): per 128-row chunk build the
row-wise Khatri-Rao product W[n, t*50+j] = Phi[n,t]*Xaug[n,j] on the vector
engine (broadcast access patterns), and accumulate
Stats[i, t*50+j] += Xaug_chunk^T @ W_chunk on the tensor engine into PSUM.
The tiny [T,41,41] inverses and all O(T*F^2) math run on the host in float64.
"""

import numpy as np

# Problem constants (hardcoded per the harness contract).
N, T, D, H = 100000, 16, 8, 40
HP1 = H + 1          # 41
F = HP1 + D + 1      # 50 Xaug columns: [hmap(40) | 1 | x(8) | y(1)]
NCORES = 8
NPC = N // NCORES    # 12500 rows per core
P = 128
NCHUNK = 100                  # padded to a multiple of DMA_BATCH
NPAD = NCHUNK * P             # 12800 (zero-padded; phi=0 rows contribute 0)
WCOLS = T * F                 # 800
HALF = WCOLS // 2             # 400 (PSUM bank limit is 512 f32 of free dim)

ALPHA_DP = 1.0
KAPPA0 = 1000.0
NU0 = 100.0
PSI0_SCALE = 500.0

TRACE = False          # set by test harness to capture an NTFF profile
LAST_RESULTS = None    # BassKernelResults of the last run (for the harness)

_CACHE = {}


DMA_BATCH = 5   # chunks per input DMA batch
ACT_T = 2       # Khatri-Rao t-blocks built by ScalarE instead of VectorE
TV = T - ACT_T  # t-blocks built on VectorE


def _build_bass(nchunk=NCHUNK, use_f32r=False):
    import concourse.tile as tile
    from concourse import bacc, mybir
    from contextlib import ExitStack

    f32 = mybir.dt.float32
    f32r = mybir.dt.float32r
    npad = nchunk * P
    assert nchunk % DMA_BATCH == 0 and nchunk % 2 == 0

    nc = bacc.Bacc("TRN2", debug=False, num_devices=NCORES)
    wdt = f32r if use_f32r else f32
    # bigc = [xaug(50) | phi(16)] per row; xaug = [hmap(40) | 1 | x(8) | y]
    # (ELM features precomputed on host). Host pre-permutes rows into
    # batch-contiguous layout [nbatch*128, DMA_BATCH*66] so each batch DMA is
    # one dense per-partition-contiguous burst.
    nbatch = nchunk // DMA_BATCH
    bigc_d = nc.dram_tensor("bigc", [nbatch * P, DMA_BATCH * (F + T)], f32,
                            kind="ExternalInput").ap()
    # accumulators: [A|B] x [even|odd PE column-group]
    out_d = nc.dram_tensor("stats", [F, 2 * WCOLS], f32,
                           kind="ExternalOutput").ap()

    with ExitStack() as ctx:
        tc = ctx.enter_context(tile.TileContext(nc))
        comb_pool = ctx.enter_context(tc.tile_pool(name="comb", bufs=5))
        w_pool = ctx.enter_context(tc.tile_pool(name="w", bufs=10))
        stats_pool = ctx.enter_context(tc.tile_pool(name="st", bufs=1,
                                                    space="PSUM"))
        out_pool = ctx.enter_context(tc.tile_pool(name="outp", bufs=1))

        # rows 0:50 accumulate even chunks (col-group 0), rows 64:114 odd
        # chunks (col-group 2) -- matmuls on distinct column-groups stream
        # concurrently through separate XBUSes.
        statsA = stats_pool.tile([64 + F, HALF], f32)
        statsB = stats_pool.tile([64 + F, HALF], f32)

        comb_t = None
        for c in range(nchunk):
            r0 = c * P
            ci = c % DMA_BATCH
            if ci == 0:
                b = c // DMA_BATCH
                comb_t = comb_pool.tile([P, DMA_BATCH, F + T], f32,
                                        tag="comb")
                nc.sync.dma_start(
                    comb_t[:],
                    bigc_d[b * P:(b + 1) * P, :].rearrange(
                        "p (c f) -> p c f", c=DMA_BATCH),
                )

            xa = comb_t[:, ci, 0:F]
            w_t = w_pool.tile([P, WCOLS], wdt)
            # W[n, t*F + j] = Phi[n, t] * Xaug[n, j]
            nc.vector.tensor_mul(
                w_t[:, 0:TV * F].rearrange("p (t f) -> p t f", t=TV),
                xa[:, None, :].broadcast_to([P, TV, F]),
                comb_t[:, ci, F:F + TV][:, :, None].broadcast_to([P, TV, F]),
            )
            for k in range(ACT_T):
                t_idx = TV + k
                nc.scalar.activation(
                    w_t[:, t_idx * F:(t_idx + 1) * F],
                    xa,
                    mybir.ActivationFunctionType.Copy,
                    scale=comb_t[:, ci, F + t_idx:F + t_idx + 1],
                )

            first, last = c < 2, c >= nchunk - 2
            g = (c % 2) * 64
            nc.tensor.matmul(statsA[g:g + F, :], xa, w_t[:, 0:HALF],
                             start=first, stop=last, tile_position=(0, g))
            nc.tensor.matmul(statsB[g:g + F, :], xa, w_t[:, HALF:WCOLS],
                             start=first, stop=last, tile_position=(0, g))

        out_t = out_pool.tile([F, 2 * WCOLS], f32)
        for gi in range(2):
            g = gi * 64
            nc.scalar.copy(out_t[:, gi * WCOLS:gi * WCOLS + HALF],
                           statsA[g:g + F, :])
            nc.vector.tensor_copy(out_t[:, gi * WCOLS + HALF:(gi + 1) * WCOLS],
                                  statsB[g:g + F, :])
        nc.sync.dma_start(out_d, out_t[:])

    nc.compile()
    return nc


def _get_bass():
    if "nc" not in _CACHE:
        _CACHE["nc"] = _build_bass()
    return _CACHE["nc"]


def _host_xaug(X, y, Wel, bel, npc, npad):
    """[hmap | 1 | x | y] in float64 -> float32 (hmap better than device ACT)."""
    Z = X.astype(np.float64) @ Wel.astype(np.float64) + \
        bel.reshape(1, H).astype(np.float64)
    hmap = 1.0 / (1.0 + np.exp(-Z))
    xa = np.zeros((npad, F), np.float32)
    xa[:npc, 0:H] = hmap.astype(np.float32)
    xa[:npc, H] = 1.0
    xa[:npc, H + 1:H + 1 + D] = X
    xa[:npc, F - 1] = y[:, 0]
    return xa


def _batch_layout(bigc):
    """[NPAD, 66] row-major -> [nbatch*128, DMA_BATCH*66] batch-contiguous."""
    nb = NCHUNK // DMA_BATCH
    fw = bigc.shape[1]
    out = bigc.reshape(nb, DMA_BATCH, P, fw).transpose(0, 2, 1, 3)
    return np.ascontiguousarray(out.reshape(nb * P, DMA_BATCH * fw))


def _make_in_maps(Phi, X, y, Wel, bel):
    in_maps = []
    for c in range(NCORES):
        sl = slice(c * NPC, (c + 1) * NPC)
        bigc = np.zeros((NPAD, F + T), np.float32)
        bigc[:, 0:F] = _host_xaug(X[sl], y[sl], Wel, bel, NPC, NPAD)
        bigc[:NPC, F:F + T] = Phi[sl]
        in_maps.append({"bigc": _batch_layout(bigc)})
    return in_maps


def _postprocess(G, epsilonA, epsilonB, zetaA, zetaB):
    """G: [T, F, F] float64 gathered Gram stats. Returns the 12-tuple."""
    epsA = np.asarray(epsilonA, np.float64)
    epsB = np.asarray(epsilonB, np.float64)
    zetA = np.asarray(zetaA, np.float64)
    zetB = np.asarray(zetaB, np.float64)

    sumPhi = G[:, H, H]                                   # [T]
    t12 = G[:, H, HP1:HP1 + D].T                          # [D, T]
    S = np.transpose(G[:, HP1:HP1 + D, HP1:HP1 + D], (1, 2, 0))  # [D,D,T]
    A = G[:, 0:HP1, 0:HP1]                                # [T,41,41]
    b = G[:, 0:HP1, F - 1]                                # [T,41]
    cst = G[:, F - 1, F - 1]                              # [T]

    BetaGamma1_new = (1.0 + sumPhi)[None, :]
    rev_incl = np.cumsum(sumPhi[::-1])[::-1]
    BetaGamma2 = (ALPHA_DP + rev_incl - sumPhi)[None, :]

    kappa = KAPPA0 + sumPhi                               # [T]
    mu = t12 / kappa[None, :]                             # [D,T]
    nu = (sumPhi + NU0)[None, :]
    psi = (PSI0_SCALE * np.eye(D)[:, :, None]
           + S
           - kappa[None, None, :] * mu[:, None, :] * mu[None, :, :])

    epsExp = (epsA / epsB)[0]                             # [T]
    zetaExp = (zetA / zetB)[0]                            # [T]
    Mm = epsExp[:, None, None] * A + zetaExp[:, None, None] * np.eye(HP1)
    WS = np.linalg.inv(Mm)                                # [T,41,41]
    WMv = zetaExp[:, None] * np.einsum('tij,tj->ti', WS, b)
    WM = WMv[:, :, None]

    zetaA_new = zetA + (H + 1) * 0.5
    zetaB_new = zetB + 0.5 * (np.sum(WMv * WMv, axis=1)
                              + np.trace(WS, axis1=1, axis2=2))[None, :]
    epsA_new = epsA + 0.5 * sumPhi[None, :]

    t1 = cst - 2.0 * np.sum(b * WMv, axis=1) + np.einsum('ti,tij,tj->t', WMv, A, WMv)
    t2 = np.einsum('tij,tji->t', A, WS)
    epsB_new = epsB + 0.5 * (t1 + t2)[None, :]

    f = lambda a: np.ascontiguousarray(np.asarray(a, np.float32))
    return (f(BetaGamma1_new), f(BetaGamma2), f(mu), f(kappa[None, :]), f(nu),
            f(psi), f(WM), f(WS), f(zetaA_new), f(zetaB_new), f(epsA_new),
            f(epsB_new))


def _ensure_ntff_hook():
    """The axon container's antenv stub lacks axon_hooks; synthesize it and
    install the ctypes NTFF profiling hook so trace=True works. Only used
    when TRACE is set (perf measurement); best-effort."""
    import sys
    import types
    try:
        import antenv.axon_hooks  # noqa: F401
        return
    except ImportError:
        pass
    try:
        import antenv
        mod = types.ModuleType("antenv.axon_hooks")
        _state = {"hook": None}
        mod.set_axon_ntff_profile_hook = lambda h: _state.__setitem__("hook", h)
        mod.get_axon_ntff_profile_hook = lambda: _state["hook"]
        sys.modules["antenv.axon_hooks"] = mod
        antenv.axon_hooks = mod
        from trn_agent_boot.trn_boot import _ntff_profile_via_ctypes
        hook = _ntff_profile_via_ctypes('/opt/axon/libaxon_pjrt.so')
        if hook is not None:
            mod.set_axon_ntff_profile_hook(hook)
    except Exception:
        pass


def kernel(BetaGamma1, Phi, inputData, outputData, weightELM, biasELM,
           epsilonA, epsilonB, zetaA, zetaB):
    global LAST_RESULTS
    if TRACE:
        _ensure_ntff_hook()
    from concourse.bass_utils import run_bass_kernel_spmd

    Phi = np.ascontiguousarray(np.asarray(Phi, np.float32))
    X = np.ascontiguousarray(np.asarray(inputData, np.float32))
    y = np.ascontiguousarray(np.asarray(outputData, np.float32))
    Wel = np.asarray(weightELM, np.float32)
    bel = np.asarray(biasELM, np.float32)

    nc = _get_bass()
    in_maps = _make_in_maps(Phi, X, y, Wel, bel)
    res = run_bass_kernel_spmd(nc, in_maps, core_ids=list(range(NCORES)),
                               trace=TRACE)
    LAST_RESULTS = res

    stats = np.zeros((F, WCOLS), np.float64)
    for r in res.results:
        s = r["stats"].astype(np.float64)       # [F, 2*WCOLS]: two col-groups
        stats += s[:, :WCOLS] + s[:, WCOLS:]
    G = stats.reshape(F, T, F).transpose(1, 0, 2)         # [t, i, j]

    return _postprocess(G, epsilonA, epsilonB, zetaA, zetaB)
